# revision 22
# baseline (speedup 1.0000x reference)
"""TRN2 Bass kernel for nn_MultiPrecisionLinear (moe_routing).

Reference computation:
    xs = x.reshape(P, bpp, S, Din)            # P=8 paths
    W  = weight_bank[assigned_bits]           # [P, Dout, Din]
    out = einsum('pbsi,poi->pbso', xs, W) + bias

Sharding: path-parallel. Core p holds path p's batch slice
[bpp*S, Din] = [32768, 256], its selected weight (as [Din, Dout]) and the
bias. All layout work happens on host so the device kernel is a pure
streaming bf16 matmul:

  x is pre-transposed, bf16-cast AND pre-chunked on host into contiguous
  blocks xt[c] = [128(i%128), 2(i//128), cw(m)] -> each DMA reads one
  contiguous block with one long contiguous run per partition.

  bf16 I/O: the correctness gate is 2e-2 rel; bf16 in / fp32-PSUM /
  bf16 out lands at ~2.6e-3 while HALVING HBM traffic vs fp32
  (33.6MB/core vs 67MB). PE rate is identical (1 cyc/row for both
  bf16 and fp32r).

  The DMA fabric is 16 engines x ~27GB/s = ~432GB/s/core; engine 79
  also serves all HWDGE ring heads and runs ~15% slower, going 100%
  busy for the whole kernel - every DMA *trigger* adds ~0.3us to its
  critical path. Hence the asymmetric chunk plan: few, huge body
  chunks (8192 cols = 4MB) to minimize trigger count, small chunks at
  the edges so the first compute starts early and the final
  in->compute->out chain is short.

  per chunk c:
    DMA in  xt[c] (Sync HWDGE ring)
    per (oc, 2048-col group): one ic-outer accumulation (2 LDWEIGHTS
    per group instead of per 512-tile) into a 4-bank PSUM tile; bias
    add fused with the PSUM->SBUF bf16 downcast, split ACT/DVE halves
    DMA out [128, 2, cw] (Scalar HWDGE ring) -> out6[c]
"""

import ml_dtypes
import numpy as np

import concourse.bacc as bacc
import concourse.mybir as mybir
import concourse.tile as tile

F32 = mybir.dt.float32
BF16 = mybir.dt.bfloat16
NP_BF16 = ml_dtypes.bfloat16
AF = mybir.ActivationFunctionType

# Problem geometry (hardcoded per spec).
P = 8          # paths == cores
BPP = 8        # batch per path
S = 4096
DIN = 256
DOUT = 256
M = BPP * S    # rows per core = 32768

# Chunk plan: small leads (fast pipeline spin-up), then uniform
# 2048-col (1MB) body chunks. Measured: coarser DMA chunks (4096/8192)
# are WORSE despite fewer triggers — fine pacing keeps the in/out
# streams smoothly interleaved on the shared DMA fabric.
PLAN = [512] * 4 + [2048] * 15
assert sum(PLAN) == M and all(cw % 512 == 0 for cw in PLAN)

_CACHE = {}


def chunk_plan():
    return list(PLAN)


def build_nc(plan=None):
    plan = chunk_plan() if plan is None else plan
    key = tuple(plan)
    if key in _CACHE:
        return _CACHE[key]

    nc = bacc.Bacc("TRN2", target_bir_lowering=False, debug=False)
    m = sum(plan)
    # xt is a flat [128, 2, m]-per-chunk sequence of contiguous blocks
    xt_d = nc.dram_tensor("xt", [128 * 2 * m], BF16, kind="ExternalInput")
    w_d = nc.dram_tensor("w", [2, 128, DOUT], BF16, kind="ExternalInput")
    bias_d = nc.dram_tensor("bias2", [2, 128], F32, kind="ExternalInput")
    out_d = nc.dram_tensor("out6", [128 * 2 * m], BF16, kind="ExternalOutput")

    with tile.TileContext(nc) as tc:
        with (
            tc.tile_pool(name="const", bufs=1) as const,
            tc.tile_pool(name="xin", bufs=4) as xin_pool,
            tc.tile_pool(name="oout", bufs=4) as oout_pool,
            tc.tile_pool(name="psum", bufs=1, space="PSUM") as psum,
        ):
            # setup DMAs on the Scalar HWDGE ring (idle early; Sync leads
            # with chunk 0)
            w_sb = const.tile([128, 2, DOUT], BF16, tag="w_sb")
            nc.scalar.dma_start(w_sb[:], w_d[:].rearrange("c p n -> p c n"))
            bias_sb = const.tile([128, 2], F32, tag="bias_sb")
            nc.scalar.dma_start(bias_sb[:], bias_d[:].rearrange("c p -> p c"))

            gi = 0   # global 2048-col group counter (PSUM tag parity)
            off = 0
            for c, cw in enumerate(plan):
                blk_in = xt_d[off : off + 128 * 2 * cw].rearrange(
                    "(p c m) -> p c m", p=128, c=2
                )
                blk_out = out_d[off : off + 128 * 2 * cw].rearrange(
                    "(p c m) -> p c m", p=128, c=2
                )
                off += 128 * 2 * cw
                xt = xin_pool.tile([128, 2, cw], BF16, name=f"xt{c}", tag=f"xt{cw}")
                nc.sync.dma_start(xt[:], blk_in)
                osb = oout_pool.tile([128, 2, cw], BF16, name=f"osb{c}", tag=f"osb{cw}")
                for oc in range(2):
                    for g0 in range(0, cw, 2048):
                        gw = min(2048, cw - g0)
                        nh = gw // 512
                        po = psum.tile(
                            [128, 2048], F32, name=f"po{c}_{oc}_{g0}",
                            tag=f"po{gi % 2}",
                        )
                        gi += 1
                        # ic-outer: one LDWEIGHTS per (oc, ic) per group
                        for ic in range(2):
                            for h in range(nh):
                                nc.tensor.matmul(
                                    po[:, h * 512 : (h + 1) * 512],
                                    w_sb[:, ic, oc * 128 : (oc + 1) * 128],
                                    xt[:, ic, g0 + h * 512 : g0 + (h + 1) * 512],
                                    start=(ic == 0),
                                    stop=(ic == 1),
                                )
                        # PSUM->SBUF with fused bias add + bf16 downcast,
                        # split between ACT and DVE so neither serializes.
                        half = (nh // 2) * 512
                        if half:
                            nc.scalar.activation(
                                osb[:, oc, g0 : g0 + half],
                                po[:, 0:half], AF.Identity,
                                bias=bias_sb[:, oc : oc + 1],
                            )
                            nc.vector.tensor_scalar_add(
                                osb[:, oc, g0 + half : g0 + gw],
                                po[:, half:gw],
                                bias_sb[:, oc : oc + 1],
                            )
                        elif gi % 2:
                            nc.scalar.activation(
                                osb[:, oc, g0 : g0 + gw], po[:, 0:gw],
                                AF.Identity, bias=bias_sb[:, oc : oc + 1],
                            )
                        else:
                            nc.vector.tensor_scalar_add(
                                osb[:, oc, g0 : g0 + gw], po[:, 0:gw],
                                bias_sb[:, oc : oc + 1],
                            )
                # Alternate output triggers between the two HWDGE rings:
                # the engines pull from both queues, keeping in/out
                # descriptors interleaved (and the drain phase, once
                # input is done, still runs on both rings).
                if c % 2:
                    nc.sync.dma_start(blk_out, osb[:])
                else:
                    nc.scalar.dma_start(blk_out, osb[:])
    nc.compile()
    _CACHE[key] = nc
    return nc


def make_in_maps(x, weight_bank, bias, assigned_bits, plan=None):
    """Host-side sharding + layout: per-core input dicts."""
    plan = chunk_plan() if plan is None else plan
    m = sum(plan)
    x = np.asarray(x, dtype=np.float32).astype(NP_BF16)
    weight_bank = np.asarray(weight_bank, dtype=np.float32)
    bias = np.asarray(bias, dtype=np.float32)
    idx = np.asarray(assigned_bits).astype(np.int64)

    bias2 = np.ascontiguousarray(bias.reshape(2, 128))
    xs = x.reshape(P, m, DIN)
    in_maps = []
    for p in range(P):
        # per chunk block[q, ic, j] = x_p[m0 + j, ic*128 + q]
        xt = np.empty(128 * 2 * m, dtype=NP_BF16)
        m0 = 0
        off = 0
        for cw in plan:
            blk = xt[off : off + 128 * 2 * cw].reshape(128, 2, cw)
            blk[:] = xs[p][m0 : m0 + cw].reshape(cw, 2, 128).transpose(2, 1, 0)
            m0 += cw
            off += 128 * 2 * cw
        w_io = np.ascontiguousarray(weight_bank[idx[p]].T).astype(NP_BF16)
        in_maps.append(
            {
                "xt": xt,
                "w": w_io.reshape(2, 128, DOUT),
                "bias2": bias2,
            }
        )
    return in_maps


def assemble_out(results, plan=None):
    plan = chunk_plan() if plan is None else plan
    m = sum(plan)
    out = np.empty((P, m, DOUT), dtype=np.float32)
    for p, r in enumerate(results):
        flat = np.asarray(r["out6"])
        m0 = 0
        off = 0
        for cw in plan:
            blk = flat[off : off + 128 * 2 * cw].reshape(128, 2, cw)
            out[p, m0 : m0 + cw] = blk.transpose(2, 1, 0).reshape(cw, DOUT).astype(np.float32)
            m0 += cw
            off += 128 * 2 * cw
    return out.reshape(P * BPP, S, DOUT)


def run_spmd_preplaced(nc, in_maps, n_cores=None):
    """Like bass2jax.run_bass_via_pjrt's multi-core path, but inputs are
    device_put + block_until_ready BEFORE launch. The stock path streams
    268MB of inputs while early cores already execute, stealing HBM
    bandwidth from them (measured: first-dispatched cores run 195-207us
    vs 173us for the last ones). Pre-placing synchronizes the start."""
    import jax
    from jax.experimental.shard_map import shard_map
    from jax.sharding import Mesh, NamedSharding, PartitionSpec

    from concourse import bass2jax
    import concourse.mybir as _mybir

    bass2jax.install_neuronx_cc_hook()
    assert nc.dbg_addr is None
    part_name = nc.partition_id_tensor.name if nc.partition_id_tensor else None

    n_cores = len(in_maps) if n_cores is None else n_cores
    in_names, out_names, out_avals, zero_shapes = [], [], [], []
    for alloc in nc.m.functions[0].allocations:
        if not isinstance(alloc, _mybir.MemoryLocationSet):
            continue
        name = alloc.memorylocations[0].name
        if alloc.kind == "ExternalInput":
            if name != part_name:
                in_names.append(name)
        elif alloc.kind == "ExternalOutput":
            out_names.append(name)
            shape = tuple(alloc.tensor_shape)
            dtype = _mybir.dt.np(alloc.dtype)
            out_avals.append(jax.core.ShapedArray(shape, dtype))
            zero_shapes.append((shape, dtype))
    n_params = len(in_names)
    n_outs = len(out_names)
    all_names = tuple(
        in_names + out_names + ([part_name] if part_name is not None else [])
    )

    def _body(*args):
        operands = list(args)
        if part_name is not None:
            operands.append(bass2jax.partition_id_tensor())
        outs = bass2jax._bass_exec_p.bind(
            *operands,
            out_avals=tuple(out_avals),
            in_names=all_names,
            out_names=tuple(out_names),
            lowering_input_output_aliases=(),
            sim_require_finite=True,
            sim_require_nnan=True,
            nc=nc,
        )
        return tuple(outs)

    devices = jax.devices()[:n_cores]
    mesh = Mesh(np.asarray(devices), ("core",))
    spec = PartitionSpec("core")
    sharded = jax.jit(
        shard_map(
            _body,
            mesh=mesh,
            in_specs=(spec,) * (n_params + n_outs),
            out_specs=(spec,) * n_outs,
            check_rep=False,
        ),
        donate_argnums=tuple(range(n_params, n_params + n_outs)),
        keep_unused=True,
    )
    concat_in = [
        np.concatenate([np.asarray(m[name]) for m in in_maps], axis=0)
        for name in in_names
    ]
    sh = NamedSharding(mesh, spec)
    placed = [jax.device_put(a, sh) for a in concat_in]
    # donated output buffers: zero-filled on device, no host transfer
    import jax.numpy as jnp

    make_zeros = jax.jit(
        lambda: tuple(
            jnp.zeros((n_cores * s[0], *s[1:]), dt) for s, dt in zero_shapes
        ),
        out_shardings=(sh,) * n_outs,
    )
    placed += list(make_zeros())
    jax.block_until_ready(placed)
    out_arrs = sharded(*placed)
    return [
        {
            name: np.asarray(out_arrs[i]).reshape(n_cores, *out_avals[i].shape)[c]
            for i, name in enumerate(out_names)
        }
        for c in range(n_cores)
    ]


def kernel(x, weight_bank, bias, assigned_bits):
    nc = build_nc()
    in_maps = make_in_maps(x, weight_bank, bias, assigned_bits)
    try:
        results = run_spmd_preplaced(nc, in_maps)
    except Exception:
        from concourse.bass_utils import run_bass_kernel_spmd

        results = run_bass_kernel_spmd(
            nc, in_maps, core_ids=list(range(P))
        ).results
    return assemble_out(results)
